# revision 1
# baseline (speedup 1.0000x reference)
"""Trainium2 Bass kernel for dual-softmax cosine-similarity attention.

Per batch b:
    pn = p / ||p||,  qn = q / ||q||           (L2 over D)
    S  = pn @ qn^T                            [L, L]
    out_p = softmax(S, axis=1) @ q            [L, D]
    out_q = softmax(S, axis=0) @ p            [L, D]

Shapes: B=64, L=512, D=768 fp32. Data-parallel over B across 8 cores
(8 batches per core).

On-chip algorithm (per batch, all layouts chosen so no on-chip input
transposes are needed -- the host supplies both [L,D] and [D,L] copies):
    G^T[j,i]  = sum_d qT[d,j] pT[d,i]            (PE, K=d)
    S^T       = G^T * rn_q[j] * rn_p[i]          (DVE + broadcast row)
    E^T       = exp(S^T), colsum[j] = sum_i E^T  (ACT, fused accum)
    F         = E^T / colsum[j]                  (DVE per-partition)
    out_p[i,:] = (E^T.T @ [q,1]) scaled by 1/rowsum (ones-column trick)
    out_q[i,:] = F.T @ p
Softmax max-subtraction is skipped: S entries are cosines in [-1,1].
The eps guard is skipped: ||x|| ~ sqrt(768) >> eps.
rsqrt is computed as exp(-0.5*ln(nsq)) to stay within one ACT table set.
"""

import numpy as np
import ml_dtypes

B, L, D = 64, 512, 768
N_CORES = 8
BPC = B // N_CORES  # batches per core
LT = L // 128  # 4
DT = D // 128  # 6

# dtype knobs: "float32r" (tf32-rate matmul, fp32 storage) or "bfloat16"
TRANS_DT = "bfloat16"  # pT/qT: operands of the similarity matmul
NAT_DT = "bfloat16"  # p/q natural + E^T/F: operands of the output matmuls

_cache = {}


def _np_dt(name):
    return ml_dtypes.bfloat16 if name == "bfloat16" else np.float32


def _build(trans_dt_name, nat_dt_name, bpc=BPC, stages="full"):
    import concourse.tile as tile
    import concourse.mybir as mybir
    from concourse import bacc
    from concourse.masks import make_identity

    f32 = mybir.dt.float32
    TD = getattr(mybir.dt, trans_dt_name)
    ND = getattr(mybir.dt, nat_dt_name)
    MUL = mybir.AluOpType.mult
    ADD = mybir.AluOpType.add
    AF = mybir.ActivationFunctionType

    nc = bacc.Bacc("TRN2", target_bir_lowering=False, debug=False)

    p_nat = nc.dram_tensor("p_nat", [bpc, L, D], ND, kind="ExternalInput").ap()
    q_nat = nc.dram_tensor("q_nat", [bpc, L, D], ND, kind="ExternalInput").ap()
    p_t = nc.dram_tensor("p_t", [bpc, D, L], TD, kind="ExternalInput").ap()
    q_t = nc.dram_tensor("q_t", [bpc, D, L], TD, kind="ExternalInput").ap()
    out_p = nc.dram_tensor("out_p", [bpc, L, D], f32, kind="ExternalOutput").ap()
    out_q = nc.dram_tensor("out_q", [bpc, L, D], f32, kind="ExternalOutput").ap()

    with tile.TileContext(nc) as tc:
        with (
            tc.tile_pool(name="singles", bufs=1) as singles,
            tc.tile_pool(name="inp", bufs=4) as inp,
            tc.tile_pool(name="ew", bufs=2) as ew,
            tc.tile_pool(name="small", bufs=2) as small,
            tc.tile_pool(name="scr", bufs=2) as scr,
            tc.tile_pool(name="outs", bufs=3) as outs,
            tc.tile_pool(name="g_ps", bufs=2, space="PSUM") as g_ps,
            tc.tile_pool(name="o_ps", bufs=2, space="PSUM") as o_ps,
            tc.tile_pool(name="row_ps", bufs=2, space="PSUM") as row_ps,
        ):
            bf16 = mybir.dt.bfloat16
            f32r = mybir.dt.float32r
            identity = singles.tile([128, 128], bf16)
            make_identity(nc, identity)
            ones_col_f = singles.tile([128, 1], f32)
            nc.vector.memset(ones_col_f, 1.0)
            ones_col = singles.tile([128, 1], ND)
            nc.vector.tensor_copy(ones_col, ones_col_f)
            ones_row = singles.tile([1, 128], bf16)
            nc.vector.memset(ones_row, 1.0)

            state = {}

            def emit_load(b, nat_first=False):
                pt = inp.tile([128, DT, L], TD, tag="pt", name=f"pt{b}")
                qt = inp.tile([128, DT, L], TD, tag="qt", name=f"qt{b}")
                pn = inp.tile([128, LT, D], ND, tag="pn", name=f"pn{b}")
                qa = inp.tile([128, LT, D], ND, tag="qa", name=f"qa{b}")

                def load_trans():
                    nc.sync.dma_start(pt, p_t[b].rearrange("(k p) n -> p k n", p=128))
                    nc.sync.dma_start(qt, q_t[b].rearrange("(k p) n -> p k n", p=128))

                def load_nat():
                    nc.sync.dma_start(
                        pn, p_nat[b].rearrange("(t p) n -> p t n", p=128)
                    )
                    nc.sync.dma_start(
                        qa, q_nat[b].rearrange("(t p) n -> p t n", p=128)
                    )

                if nat_first:
                    load_nat()
                    load_trans()
                else:
                    load_trans()
                    load_nat()
                state[b] = dict(pt=pt, qt=qt, pn=pn, qa=qa)

            def emit_norms_pre(b):
                """Squared-norm -> rsqrt chain, no PE instructions.
                p-squares on ACT (Square + fused accumulate), q-squares on
                GpSimd (otherwise idle) + DVE reduce, rsqrt via bit-trick
                seed + 3 Newton steps on DVE (keeps ACT free of Ln/Sqrt)."""
                st = state[b]
                pn, qa = st["pn"], st["qa"]
                i32 = mybir.dt.int32
                # nsq cols 0..3 = p tiles, 4..7 = q tiles
                nsq = small.tile([128, 2 * LT], f32, tag="nsq", name=f"nsq{b}")
                for t in range(LT):
                    s1 = scr.tile([128, D], bf16, tag="scr", name=f"sp{b}_{t}")
                    nc.scalar.activation(
                        s1, pn[:, t, :], AF.Square, accum_out=nsq[:, t : t + 1]
                    )
                    s2 = scr.tile([128, D], bf16, tag="scr", name=f"sq{b}_{t}")
                    nc.gpsimd.tensor_mul(s2, qa[:, t, 0:D], qa[:, t, 0:D])
                    nc.vector.reduce_sum(
                        nsq[:, LT + t : LT + t + 1], s2, axis=mybir.AxisListType.X
                    )
                # rsqrt: y0 = bits(0x5f3759df - (bits(x) >> 1)), then Newton
                W = 2 * LT
                yi = small.tile([128, W], i32, tag="yi", name=f"yi{b}")
                nc.vector.tensor_scalar(
                    yi,
                    nsq.bitcast(i32),
                    scalar1=1,
                    scalar2=None,
                    op0=mybir.AluOpType.arith_shift_right,
                )
                nc.vector.tensor_scalar(
                    yi, yi, scalar1=-1, scalar2=0x5F3759DF, op0=MUL, op1=ADD
                )
                rn = small.tile([128, W], f32, tag="rn", name=f"rn{b}")
                nc.vector.tensor_copy(rn, yi.bitcast(f32))
                t1 = small.tile([128, W], f32, tag="t1", name=f"t1{b}")
                for _ in range(1):
                    nc.vector.tensor_mul(t1, rn, rn)
                    nc.vector.tensor_mul(t1, t1, nsq)
                    nc.vector.tensor_scalar(
                        t1, t1, scalar1=-0.5, scalar2=1.5, op0=MUL, op1=ADD
                    )
                    nc.vector.tensor_mul(rn, rn, t1)
                st["rn"] = rn

            def emit_norms_pe(b):
                """rn_p columns -> [1, 512] row (PE transposes, bf16) ->
                broadcast to 128 partitions (K=1 ones matmul)."""
                st = state[b]
                rn = st["rn"]
                # bf16 copy of the rn_p columns keeps the PE in 1-pass
                # bf16 mode (no fp32 LOW_HIGH transposes)
                rnb = small.tile([128, LT], bf16, tag="rnb", name=f"rnb{b}")
                nc.vector.tensor_copy(rnb, rn[:, 0:LT])
                rowp = row_ps.tile([1, L], bf16, tag="rowp", bufs=1, name=f"rowp{b}")
                for t in range(LT):
                    nc.tensor.transpose(
                        rowp[0:1, t * 128 : (t + 1) * 128], rnb[:, t : t + 1], identity
                    )
                rowf = small.tile([1, L], bf16, tag="rowf", name=f"rowf{b}")
                nc.vector.tensor_copy(rowf, rowp)
                bc = g_ps.tile([128, L], f32, tag="g", name=f"bc{b}")
                nc.tensor.matmul(bc, lhsT=ones_row, rhs=rowf)
                rnp = small.tile([128, L], f32, tag="rnp", name=f"rnp{b}")
                nc.scalar.copy(rnp, bc)
                st["rnp"] = rnp

            def emit_gexp(b, hook=None):
                st = state[b]
                pt, qt = st["pt"], st["qt"]
                et = ew.tile([128, LT, L], ND, tag="et", name=f"et{b}")
                f = ew.tile([128, LT, L], ND, tag="f", name=f"f{b}")
                colsum = small.tile([128, LT], f32, tag="colsum", name=f"cs{b}")
                rcol = small.tile([128, LT], f32, tag="rcol", name=f"rc{b}")
                for jt in range(LT):
                    gp = g_ps.tile([128, L], f32, tag="g", name=f"g{b}_{jt}")
                    for kt in range(DT):
                        nc.tensor.matmul(
                            gp,
                            lhsT=qt[:, kt, jt * 128 : (jt + 1) * 128],
                            rhs=pt[:, kt, :],
                            start=(kt == 0),
                            stop=(kt == DT - 1),
                        )
                    if jt == 0 and hook is not None:
                        hook()
                    rn, rnp = st["rn"], st["rnp"]
                    stt = scr.tile([128, L], f32, tag="stt", name=f"stt{b}_{jt}")
                    nc.vector.tensor_mul(stt, gp, rnp)
                    nc.scalar.activation(
                        et[:, jt, :],
                        stt,
                        AF.Exp,
                        scale=rn[:, LT + jt : LT + jt + 1],
                        accum_out=colsum[:, jt : jt + 1],
                    )
                nc.vector.reciprocal(rcol, colsum)
                for jt in range(LT):
                    nc.vector.tensor_scalar_mul(
                        f[:, jt, :], et[:, jt, :], rcol[:, jt : jt + 1]
                    )
                # rowsum[i] = sum_j E^T[j,i] as a [1, L] row via ones matmul,
                # then 1/rowsum broadcast to all partitions and folded into
                # the out_p weights (ep) -- keeps out_p structurally
                # identical to out_q
                rs_ps = row_ps.tile([1, L], f32, tag="rs", bufs=1, name=f"rs{b}")
                for jt in range(LT):
                    nc.tensor.matmul(
                        rs_ps,
                        lhsT=ones_col,
                        rhs=et[:, jt, :],
                        start=(jt == 0),
                        stop=(jt == LT - 1),
                    )
                rrow = small.tile([1, L], f32, tag="rrow", name=f"rr{b}")
                nc.vector.reciprocal(rrow, rs_ps)
                rrow_b = small.tile([1, L], bf16, tag="rrow_b", name=f"rrb{b}")
                nc.vector.tensor_copy(rrow_b, rrow)
                st["rrow_b"] = rrow_b
                st["et"] = et
                st["f"] = f

            def emit_ep(b):
                """Broadcast 1/rowsum and fold it into the out_p weights.
                Emitted after out(b-1) so the PE never waits on the DVE
                reciprocal chain."""
                st = state[b]
                et, rrow_b = st["et"], st["rrow_b"]
                rsb_ps = g_ps.tile([128, L], f32, tag="g", name=f"rsbp{b}")
                nc.tensor.matmul(rsb_ps, lhsT=ones_row, rhs=rrow_b)
                rsb = small.tile([128, L], f32, tag="rsb", name=f"rsb{b}")
                nc.scalar.copy(rsb, rsb_ps)
                ep = ew.tile([128, LT, L], ND, tag="ep", name=f"ep{b}")
                for jt in range(LT):
                    nc.gpsimd.tensor_mul(ep[:, jt, :], et[:, jt, :], rsb)
                st["ep"] = ep

            def emit_one_out(lhs, rhs, dram, b, m, tag, evac):
                mm = slice(m * 128, (m + 1) * 128)
                ps = o_ps.tile([128, D], f32, tag="ops", name=f"{tag}{b}_{m}")
                for jt in range(LT):
                    nc.tensor.matmul(
                        ps[:, 0:512],
                        lhsT=lhs[:, jt, mm],
                        rhs=rhs[:, jt, 0:512],
                        start=(jt == 0),
                        stop=(jt == LT - 1),
                    )
                for jt in range(LT):
                    nc.tensor.matmul(
                        ps[:, 512:D],
                        lhsT=lhs[:, jt, mm],
                        rhs=rhs[:, jt, 512:D],
                        start=(jt == 0),
                        stop=(jt == LT - 1),
                    )
                sb = outs.tile([128, D], f32, tag=f"{tag}_sb", name=f"{tag}s{b}_{m}")
                if evac == "act":
                    nc.scalar.copy(sb, ps[:, 0:D])
                else:
                    nc.vector.tensor_copy(sb, ps[:, 0:D])
                nc.sync.dma_start(dram[b, mm, :], sb)

            def emit_out(b):
                st = state[b]
                pn, qa, ep, f = st["pn"], st["qa"], st["ep"], st["f"]
                for m in range(LT):
                    emit_one_out(ep, qa, out_p, b, m, "op", "act")
                    emit_one_out(f, pn, out_q, b, m, "oq", "dve")

            # software pipeline: output matmuls for batch b-1 are emitted
            # after batch b's similarity matmuls so the PE never waits on
            # the exp/scale chain of the current batch
            # Software pipeline. Per step b the PE stream is
            #   G-matmuls(b) | out-matmuls(b-1) | transposes(b+1)
            # and loads run two batches ahead, so the exp/F chain of batch b
            # and the norm chain of b+1 complete on DVE/ACT/GpSimd before
            # their consumers reach the head of the PE queue.
            emit_load(0, nat_first=True)
            for nb in (1, 2):
                if nb < bpc:
                    emit_load(nb, nat_first=(nb == 1))
            emit_norms_pre(0)
            for b in range(bpc):
                if b == 0:
                    # b=0: splice the transposes after the first G group so
                    # the PE starts on G as soon as pt/qt arrive
                    emit_gexp(0, hook=lambda: emit_norms_pe(0))
                else:
                    emit_gexp(b)
                if b > 0:
                    emit_out(b - 1)
                emit_ep(b)
                if b + 3 < bpc:
                    emit_load(b + 3)
                if b + 1 < bpc:
                    emit_norms_pre(b + 1)
                    emit_norms_pe(b + 1)
            emit_out(bpc - 1)

    nc.compile()
    return nc


def _get_nc():
    key = (TRANS_DT, NAT_DT)
    if key not in _cache:
        _cache[key] = _build(*key)
    return _cache[key]


def kernel(p, q):
    from concourse.bass_utils import run_bass_kernel_spmd

    nc = _get_nc()
    p = np.asarray(p)
    q = np.asarray(q)
    td = _np_dt(TRANS_DT)
    nd = _np_dt(NAT_DT)

    in_maps = []
    for c in range(N_CORES):
        sl = slice(c * BPC, (c + 1) * BPC)
        ps, qs = p[sl], q[sl]
        in_maps.append(
            {
                "p_nat": np.ascontiguousarray(ps).astype(nd),
                "q_nat": np.ascontiguousarray(qs).astype(nd),
                "p_t": np.ascontiguousarray(ps.transpose(0, 2, 1)).astype(td),
                "q_t": np.ascontiguousarray(qs.transpose(0, 2, 1)).astype(td),
            }
        )

    res = run_bass_kernel_spmd(nc, in_maps, core_ids=list(range(N_CORES)))
    _cache["last_result"] = res
    vec_att_p = np.concatenate([r["out_p"] for r in res.results], axis=0)
    vec_att_q = np.concatenate([r["out_q"] for r in res.results], axis=0)
    return vec_att_p, vec_att_q


if __name__ == "__main__":
    rng = np.random.default_rng(0)
    p = rng.standard_normal((B, L, D)).astype(np.float32)
    q = rng.standard_normal((B, L, D)).astype(np.float32)
    op, oq = kernel(p, q)
    print("shapes:", op.shape, oq.shape, op.dtype, oq.dtype)

